# revision 55
# baseline (speedup 1.0000x reference)
"""Block-sparse attention (local + vertical-strided causal mask) on 8 TRN2 cores.

Sharding: one head per NeuronCore (H=8, n_cores=8).

Per-core device algorithm (head h, residue r = 7-h):
  The 4096x4096 score matrix is processed at 128x128 granularity:
  "pair" i = q block-rows (2i, 2i+1) (128 q tokens), "chunk" = 128 k tokens
  (2 mask blocks of 64). Local window -> chunks c in [i-8, i] of K itself;
  vertical-strided blocks -> host-gathered K_vert (6 blocks of 64, kb = 8j+r),
  processed as 3 chunks shared by all cores, with per-core validity applied
  as multiplicative 0/1 per-partition scalars.

  S^T orientation: S^T[k,q] = kT_chunk.T @ qT_pair  (PE, bf16; sm_scale
  pre-baked into K on the host so exp takes no scale operand)
  P^T = exp(S^T)                                     (ACT, one call per group)
  masks: triangle / window-start / per-visit vert-validity tiles, one
  full-tile multiply each on DVE (vert visits valid on every core skip
  the mask entirely)
  out[q,0:128] += P^T_chunk.T @ [V | 1]_chunk        (PE, PSUM-accumulated)
  col 128 of out = softmax denominator; epilogue reads PSUM directly:
  strided DVE reciprocal + per-pair tensor_scalar multiply into a
  [128, 3, D] staging tile, one batched store per pair-group into a
  partition-major DRAM output o[p, i, d] (host restores [S, D] order).
"""

import numpy as np
import ml_dtypes

BF16 = ml_dtypes.bfloat16

H = 8
S = 4096
D = 128
BLK = 64
NB = S // BLK        # 64 block rows
NPAIR = NB // 2      # 32 row pairs
NVSLOT = 6           # usable vertical slots (kb = 8j + r <= 47)
NVC = NVSLOT // 2    # 3 vertical chunks
GROUP = 8            # PSUM staging slots per exp group (8 * 128 f32 = 2 banks)
PG = 3               # pairs per oacc tile (1 PSUM bank)

NEG = -30000.0


def make_schedule():
    """Global ordered visit list, chunk-major. visit = (kind, idx, pair)
    kind "local": idx = chunk c (k blocks 2c, 2c+1), pairs i in [c, c+8]
    kind "vert":  idx = vc (K_vert slots 2vc, 2vc+1)
    Vert visits for pairs [c0, c0+8) are inserted right before local chunk
    c0 in {8, 16, 24}, after all their opening local chunks."""
    visits = []
    for c in range(NPAIR):
        if c in (8, 16, 24):
            for vc in range(NVC):
                if 8 * vc + 8 <= c:
                    for i in range(c, c + 8):
                        visits.append(("vert", vc, i))
        for i in range(c, min(c + 8, NPAIR - 1) + 1):
            visits.append(("local", c, i))
    return visits


def vert_visit_order():
    return [(vc_, i_) for (kind, vc_, i_) in make_schedule() if kind == "vert"]


def vert_all_valid(vc, i):
    """True iff the (vc, i) vert visit is unmasked on EVERY core (r=0..7):
    highest slot kb = 16vc + 8 + r <= qb_min - 16 = 2i - 16 for r=7."""
    return 16 * vc + 15 <= 2 * i - 16


def vert_mask_slots():
    """Mask-table slot index per masked vert visit, keyed by (vc, i)."""
    slots = {}
    for vc, i in vert_visit_order():
        if not vert_all_valid(vc, i):
            slots[(vc, i)] = len(slots)
    return slots


_PROGRAM = None


def _build_program(loop_n=None, ablate=(), pv_delay=3, group=GROUP,
                   pt_bufs=None, exp_split=1, stage_bufs=2, oacc_bufs=4,
                   mask_eng="dve", store_eng="sp", ob_bufs=3, rd_bufs=4):
    if pt_bufs is None:
        pt_bufs = pv_delay + 2
    """Build the SPMD program. loop_n: wrap the whole body (incl. input DMA)
    in an in-NEFF For loop with that trip count — used only for timing.
    ablate: subset of {"masks","pv","epi","exp"} — drop stages (timing only).
    pv_delay: groups of software-pipeline delay between S^T and PV."""
    import contextlib
    import concourse.bass as bass
    import concourse.mybir as mybir
    import concourse.tile as tile
    from concourse import bacc

    fp32 = mybir.dt.float32
    bf16 = mybir.dt.bfloat16

    nc = bacc.Bacc("TRN2", target_bir_lowering=False, debug=False, num_devices=H)

    qt_d = nc.dram_tensor("qt", [D, S], bf16, kind="ExternalInput").ap()
    kt_d = nc.dram_tensor("kt", [D, S], bf16, kind="ExternalInput").ap()
    ktv_d = nc.dram_tensor("ktv", [D, NVSLOT * BLK], bf16, kind="ExternalInput").ap()
    vaug_d = nc.dram_tensor("vaug", [128, NPAIR, D + 1], bf16, kind="ExternalInput").ap()
    vvaug_d = nc.dram_tensor("vvaug", [128, NVC, D + 1], bf16, kind="ExternalInput").ap()
    nvm = len(vert_mask_slots())
    vm_d = nc.dram_tensor("vm", [128, nvm, 128], bf16, kind="ExternalInput").ap()
    tri_d = nc.dram_tensor("tri", [128, 128], bf16, kind="ExternalInput").ap()
    mstart_d = nc.dram_tensor("mstart", [128, 128], bf16, kind="ExternalInput").ap()
    # partition-major output: o[p, i, d] = out[i*128 + p, d]; host transposes
    o_d = nc.dram_tensor("o", [128, NPAIR, D], fp32, kind="ExternalOutput").ap()

    visits = make_schedule()
    # first/last visit index per pair
    first = {}
    last = {}
    for g, (kind, idx, i) in enumerate(visits):
        first.setdefault(i, g)
        last[i] = g
    # PSUM start_tensor_calc zeroes the full 2KB bank (zero-region), so only
    # the first matmul touching an oacc tile may carry start=True.
    tile_first = {}
    for g, (kind, idx, i) in enumerate(visits):
        tile_first.setdefault(i // 3, g)
    with tile.TileContext(nc) as tc:
        with (
            tc.tile_pool(name="big", bufs=1) as big,
            tc.tile_pool(name="stage", bufs=stage_bufs, space="PSUM") as stagep,
            tc.tile_pool(name="oacc", bufs=oacc_bufs, space="PSUM") as oaccp,
            tc.tile_pool(name="pt", bufs=pt_bufs) as ptp,
            tc.tile_pool(name="ob", bufs=ob_bufs) as obp,
            tc.tile_pool(name="rd", bufs=rd_bufs) as rdp,
        ):
            if loop_n is not None:
                loop_cm = tc.For_i(
                    0,
                    loop_n,
                    hint_engines=(
                        mybir.EngineType.PE,
                        mybir.EngineType.DVE,
                        mybir.EngineType.Activation,
                        mybir.EngineType.Pool,
                        mybir.EngineType.SP,
                    ),
                )
            else:
                loop_cm = contextlib.nullcontext()
            with loop_cm:
                _emit_body(nc, tc, locals(), frozenset(ablate),
                           pv_delay=pv_delay, group=group, exp_split=exp_split,
                           mask_eng=mask_eng, store_eng=store_eng)
    nc.compile()
    return nc


def _emit_body(nc, tc, env, ablate=frozenset(), pv_delay=1, group=GROUP,
               exp_split=1, mask_eng="pool", store_eng="sp"):
    GROUP = group
    import concourse.mybir as mybir

    fp32 = mybir.dt.float32
    bf16 = mybir.dt.bfloat16
    big, stagep, oaccp, ptp, obp, rdp = (
        env["big"], env["stagep"], env["oaccp"], env["ptp"], env["obp"], env["rdp"]
    )
    qt_d, kt_d, ktv_d, vaug_d, vvaug_d, vm_d, tri_d, mstart_d, o_d = (
        env["qt_d"], env["kt_d"], env["ktv_d"], env["vaug_d"], env["vvaug_d"],
        env["vm_d"], env["tri_d"], env["mstart_d"], env["o_d"],
    )
    vm_slots = vert_mask_slots()
    visits, first, last, tile_first = (
        env["visits"], env["first"], env["last"], env["tile_first"]
    )
    n_groups = (len(visits) + GROUP - 1) // GROUP

    qt = big.tile([D, S], bf16)
    kt = big.tile([D, S], bf16)
    ktv = big.tile([D, NVSLOT * BLK], bf16)
    vaug = big.tile([128, NPAIR, D + 1], bf16)
    vvaug = big.tile([128, NVC, D + 1], bf16)
    vm = big.tile([128, len(vert_mask_slots()), 128], bf16)
    tri = big.tile([128, 128], bf16)
    mstart = big.tile([128, 128], bf16)

    mask_e = nc.gpsimd if mask_eng == "pool" else nc.vector
    store_e = nc.sync if store_eng == "sp" else nc.gpsimd

    # Input DMAs spread over the three DMA-capable queues.  Transfers
    # serialize per queue, so each queue is ordered by first use; the
    # group-0-gating pieces (qt lead slices on SP, kt lead slices on ACT,
    # tri via Pool) resolve ~2.5-3.5us in.
    nc.sync.dma_start(out=qt[:, 0:256], in_=qt_d[:, 0:256])
    nc.sync.dma_start(out=qt[:, 256:512], in_=qt_d[:, 256:512])
    nc.sync.dma_start(out=qt[:, 512:1024], in_=qt_d[:, 512:1024])
    nc.sync.dma_start(out=qt[:, 1024:2048], in_=qt_d[:, 1024:2048])
    nc.sync.dma_start(out=kt[:, 512:1536], in_=kt_d[:, 512:1536])
    nc.sync.dma_start(out=qt[:, 2048:4096], in_=qt_d[:, 2048:4096])
    nc.sync.dma_start(out=kt[:, 1536:2816], in_=kt_d[:, 1536:2816])
    nc.sync.dma_start(out=kt[:, 2816:4096], in_=kt_d[:, 2816:4096])
    nc.sync.dma_start(out=vm[:], in_=vm_d[:])
    nc.sync.dma_start(out=ktv[:], in_=ktv_d[:])
    nc.sync.dma_start(out=vvaug[:], in_=vvaug_d[:])
    # Pool SWDGE: small group-0/1 tensors (tri gates masks(0)).
    nc.gpsimd.dma_start(out=tri[:], in_=tri_d[:])
    nc.gpsimd.dma_start(out=mstart[:], in_=mstart_d[:])
    # ACT HWDGE: the chunk-0 kt slice first (gates the first S^T — a tiny
    # 128-col lead slice so its descriptor+transfer clears fast), vaug
    # staged in thirds (first use is PV, pv_delay groups in), later slices
    # emitted inside the group loop so they don't delay exp(0).
    nc.scalar.dma_start(out=kt[:, 0:128], in_=kt_d[:, 0:128])
    nc.scalar.dma_start(out=kt[:, 128:512], in_=kt_d[:, 128:512])
    nc.scalar.dma_start(out=vaug[:, 0:8], in_=vaug_d[:, 0:8])

    oacc_tiles = {}  # pair-group -> psum tile
    pending_pv = []  # software pipeline: PV of group gi-d emitted
    # after S^T of group gi so PE streams while ACT/DVE process gi-d

    for gi in range(n_groups):
        if gi == 1:
            nc.scalar.dma_start(out=vaug[:, 8:16], in_=vaug_d[:, 8:16])
        elif gi == 3:
            nc.scalar.dma_start(out=vaug[:, 16:NPAIR], in_=vaug_d[:, 16:NPAIR])
        gvis = visits[gi * GROUP : (gi + 1) * GROUP]
        n = len(gvis)
        stage = stagep.tile([128, GROUP * 128], fp32, tag="stage")
        ptt = ptp.tile([128, GROUP * 128], bf16, tag="pt")

        # --- S^T matmuls, batched over runs of consecutive pairs
        # sharing one k-chunk, split at 4-slot (one PSUM bank) bounds.
        # start=True only on the first run per bank (bank zero-region).
        s = 0
        seen_banks = set()
        while s < n:
            kind, idx, i0 = gvis[s]
            e = s + 1
            while (
                e < n
                and e % 4 != 0
                and not (gi == 0 and e == 2)
                and gvis[e][0] == kind
                and gvis[e][1] == idx
                and gvis[e][2] == gvis[e - 1][2] + 1
            ):
                e += 1
            ln = e - s
            lhsT = (
                kt[:, idx * 128 : (idx + 1) * 128]
                if kind == "local"
                else ktv[:, idx * 128 : (idx + 1) * 128]
            )
            bank = s // 4
            nc.tensor.matmul(
                stage[:, s * 128 : e * 128],
                lhsT,
                qt[:, i0 * 128 : (i0 + ln) * 128],
                start=bank not in seen_banks,
                stop=True,
                skip_group_check=True,
            )
            if "dup_st" in ablate:
                nc.tensor.matmul(
                    stage[:, s * 128 : e * 128],
                    lhsT,
                    qt[:, i0 * 128 : (i0 + ln) * 128],
                    start=False,
                    stop=True,
                    skip_group_check=True,
                )
            seen_banks.add(bank)
            s = e

        # software pipeline with end-taper: drain pending PV batches faster
        # near the tail so they overlap the last exps instead of following.
        remaining = n_groups - 1 - gi
        while len(pending_pv) > min(pv_delay - 1, max(0, remaining - 1)):
            pending_pv.pop(0)()

        # --- exp for the group (group 0 split finely so the first call only
        # needs the first two S^T slots -> earlier ACT rampup)
        if "exp" not in ablate:
            if gi == 0:
                bounds = [(0, 2), (2, 4), (4, n)]
            else:
                per = (n + exp_split - 1) // exp_split
                bounds = [(es, min(n, es + per)) for es in range(0, n, per)]
            for es, ee in bounds:
                for _rep in range(2 if "dup_exp" in ablate else 1):
                    nc.scalar.activation(
                        out=ptt[:, es * 128 : ee * 128],
                        in_=stage[:, es * 128 : ee * 128],
                        func=mybir.ActivationFunctionType.Exp,
                    )

        # --- masks (tri/mstart on mask_e; vert validity = one full-tile
        # multiply on DVE, and only for visits not valid on every core)
        for s, (kind, idx, i) in enumerate(gvis):
            if "masks" in ablate:
                continue
            sl = slice(s * 128, (s + 1) * 128)
            reps = 2 if "dup_masks" in ablate else 1
            if kind == "local" and idx == i:
                for _rep in range(reps):
                    mask_e.tensor_mul(ptt[:, sl], ptt[:, sl], tri[:])
            elif kind == "local" and idx == i - 8:
                for _rep in range(reps):
                    mask_e.tensor_mul(ptt[:, sl], ptt[:, sl], mstart[:])
            elif kind == "vert" and (idx, i) in vm_slots:
                for _rep in range(reps):
                    nc.vector.tensor_mul(
                        ptt[:, sl], ptt[:, sl], vm[:, vm_slots[(idx, i)]]
                    )

        # --- PV matmuls + epilogue (deferred pv_delay groups)
        # oacc tile = [128, 2, 512] f32 (2 banks); pair slot jj = i % PG sits
        # at bank jj//3, cols (jj%3)*129 .. +129 so no slot crosses a bank.
        def pslot(oacc, i, a, b):
            jj = i % PG
            c0 = (jj % 3) * 129
            return oacc[:, jj // 3, c0 + a : c0 + b]

        def make_pv(gi, gvis, ptt):
            def emit_pv():
                if "pv" in ablate:
                    return
                for s, (kind, idx, i) in enumerate(gvis):
                    g = gi * GROUP + s
                    pg = i // PG
                    if pg not in oacc_tiles:
                        oacc_tiles[pg] = oaccp.tile(
                            [128, (PG + 2) // 3, 512], fp32, tag="oacc",
                            name=f"oacc{pg}"
                        )
                    oacc = oacc_tiles[pg]
                    rhs = vaug[:, idx] if kind == "local" else vvaug[:, idx]
                    nc.tensor.matmul(
                        pslot(oacc, i, 0, D + 1),
                        ptt[:, s * 128 : (s + 1) * 128],
                        rhs,
                        start=(g == tile_first[i // 3]),
                        stop=(g == last[i]) and "dup_pv" not in ablate,
                        skip_group_check=True,
                    )
                    if "dup_pv" in ablate:
                        nc.tensor.matmul(
                            pslot(oacc, i, 0, D + 1),
                            ptt[:, s * 128 : (s + 1) * 128],
                            rhs,
                            start=False,
                            stop=(g == last[i]),
                            skip_group_check=True,
                        )
                    # epilogue once per oacc tile (after its last pair
                    # closes): per-pair reciprocal of the PSUM denominator
                    # column + tensor_scalar multiply straight out of PSUM
                    # into a [128, PG, D] staging tile, one batched store.
                    pg_pairs = list(range(PG * pg, min(PG * (pg + 1), NPAIR)))
                    last_pg = (NPAIR - 1) // PG
                    if "epi" in ablate:
                        pass
                    elif pg == last_pg and g == last[i]:
                        # tail: per-pair epilogue+store as each pair closes,
                        # so the final store chain is one pair deep.
                        ob = obp.tile([128, 1, D], fp32, tag="obt")
                        rd = rdp.tile([128, 1], fp32, tag="rd")
                        nc.vector.reciprocal(rd[:], pslot(oacc, i, D, D + 1))
                        nc.vector.tensor_scalar_mul(
                            ob[:, 0], pslot(oacc, i, 0, D), rd[:]
                        )
                        (nc.scalar if i == NPAIR - 1 else store_e).dma_start(
                            out=o_d[:, i : i + 1, :], in_=ob[:]
                        )
                    elif i == pg_pairs[-1] and g == last[i]:
                        npp = len(pg_pairs)
                        ob = obp.tile([128, PG, D], fp32, tag="ob")
                        rd = rdp.tile([128, PG], fp32, tag="rd")
                        # one strided reciprocal covers all denominators
                        nc.vector.reciprocal(
                            rd[:, 0:npp],
                            oacc[:, 0, D : D + (npp - 1) * 129 + 1 : 129],
                        )
                        for jj, p in enumerate(pg_pairs):
                            nc.vector.tensor_scalar_mul(
                                ob[:, jj], pslot(oacc, p, 0, D),
                                rd[:, jj : jj + 1]
                            )
                        store_e.dma_start(
                            out=o_d[:, PG * pg : PG * pg + npp, :],
                            in_=ob[:, 0:npp],
                        )
            return emit_pv

        pending_pv.append(make_pv(gi, gvis, ptt))
    for f in pending_pv:
        f()


def _get_program():
    global _PROGRAM
    if _PROGRAM is None:
        _PROGRAM = _build_program()
    return _PROGRAM


def _host_inputs(q, k, v, sm_scale):
    """Per-core input dicts (host-side shard + layout)."""
    q = np.asarray(q, dtype=np.float32)
    k = np.asarray(k, dtype=np.float32)
    v = np.asarray(v, dtype=np.float32)
    smv = float(np.asarray(sm_scale, dtype=np.float32))

    tri = np.zeros((128, 128), dtype=BF16)
    p = np.arange(128)
    tri[p[:, None] <= p[None, :]] = BF16(1.0)
    mstart = np.zeros((128, 128), dtype=BF16)
    mstart[64:, :64] = BF16(1.0)

    vm_slots = vert_mask_slots()
    ins = []
    for h in range(H):
        r = 7 - h
        qh, kh, vh = q[0, h], k[0, h], v[0, h]
        qt = np.ascontiguousarray(qh.T).astype(BF16)
        # sm_scale baked into K so the device exp needs no scale operand
        kt = np.ascontiguousarray(kh.T * smv).astype(BF16)
        vblocks = [8 * j + r for j in range(NVSLOT)]
        kv = np.concatenate([kh[b * BLK : (b + 1) * BLK] for b in vblocks], axis=0)
        ktv = np.ascontiguousarray(kv.T * smv).astype(BF16)
        vaug = np.concatenate(
            [vh, np.ones((S, 1), np.float32)], axis=1
        ).astype(BF16)  # [4096, 129]
        vaug = np.ascontiguousarray(
            vaug.reshape(NPAIR, 128, D + 1).transpose(1, 0, 2)
        )  # [128, 32, 129]
        vv = np.concatenate([vh[b * BLK : (b + 1) * BLK] for b in vblocks], axis=0)
        vvaug = np.concatenate([vv, np.ones((NVSLOT * BLK, 1), np.float32)], axis=1)
        vvaug = np.ascontiguousarray(
            vvaug.astype(BF16).reshape(NVC, 128, D + 1).transpose(1, 0, 2)
        )  # [128, 3, 129]

        # per-visit full vert-validity mask tiles (only for visits that are
        # not all-valid on every core): vm[k, slot, q] = kb(k) <= qb(q) - 16
        vmt = np.zeros((128, len(vm_slots), 128), dtype=BF16)
        for (vc, i), si in vm_slots.items():
            kb = 8 * (2 * vc + (p >= 64).astype(np.int64)) + r      # [128] per k
            qb = 2 * i + (p >= 64).astype(np.int64)                 # [128] per q
            vmt[:, si, :] = (kb[:, None] <= qb[None, :] - 16).astype(BF16)
        ins.append(
            dict(
                qt=qt, kt=kt, ktv=ktv, vaug=vaug, vvaug=vvaug,
                vm=vmt, tri=tri, mstart=mstart,
            )
        )
    return ins


def kernel(q, k, v, sm_scale):
    from concourse.bass_utils import run_bass_kernel_spmd

    nc = _get_program()
    ins = _host_inputs(q, k, v, sm_scale)
    res = run_bass_kernel_spmd(nc, ins, core_ids=list(range(H)))
    # device output is partition-major o[p, i, d]; restore [S, D]
    out = np.stack(
        [
            res.results[h]["o"].transpose(1, 0, 2).reshape(S, D)
            for h in range(H)
        ],
        axis=0,
    )[None]
    return out.astype(np.float32)


# revision 69
# speedup vs baseline: 1.1444x; 1.1444x over previous
"""Block-sparse attention (local + vertical-strided causal mask) on 8 TRN2 cores.

Sharding: one head per NeuronCore (H=8, n_cores=8).

Per-core device algorithm (head h, residue r = 7-h):
  The 4096x4096 score matrix is processed at 128x128 granularity:
  "pair" i = q block-rows (2i, 2i+1) (128 q tokens), "chunk" = 128 k tokens
  (2 mask blocks of 64). Local window -> chunks c in [i-8, i] of K itself;
  vertical-strided blocks -> host-gathered K_vert (6 blocks of 64, kb = 8j+r),
  processed as 3 chunks shared by all cores, with per-core validity applied
  as multiplicative 0/1 per-partition scalars.

  S^T orientation: S^T[k,q] = kT_chunk.T @ qT_pair  (PE, bf16; sm_scale
  pre-baked into K on the host so exp takes no scale operand)
  P^T = exp(S^T)                                     (ACT, one call per group)
  masks: triangle / window-start / per-visit vert-validity tiles, one
  full-tile multiply each on DVE (vert visits valid on every core skip
  the mask entirely)
  out[q,0:128] += P^T_chunk.T @ [V | 1]_chunk        (PE, PSUM-accumulated)
  col 128 of out = softmax denominator; epilogue reads PSUM directly:
  strided DVE reciprocal + per-pair tensor_scalar multiply into a
  [128, 3, D] staging tile, one batched store per pair-group into a
  partition-major DRAM output o[p, i, d] (host restores [S, D] order).
"""

import numpy as np
import ml_dtypes

BF16 = ml_dtypes.bfloat16

H = 8
S = 4096
D = 128
BLK = 64
NB = S // BLK        # 64 block rows
NPAIR = NB // 2      # 32 row pairs
NVSLOT = 6           # usable vertical slots (kb = 8j + r <= 47)
NVC = NVSLOT // 2    # 3 vertical chunks
GROUP = 8            # PSUM staging slots per exp group (8 * 128 f32 = 2 banks)
PG = 3               # pairs per oacc tile (1 PSUM bank)

NEG = -30000.0


def make_schedule():
    """Global ordered visit list, chunk-major. visit = (kind, idx, pair)
    kind "local": idx = chunk c (k blocks 2c, 2c+1), pairs i in [c, c+8]
    kind "vert":  idx = vc (K_vert slots 2vc, 2vc+1)
    Vert visits for pairs [c0, c0+8) are inserted right before local chunk
    c0 in {8, 16, 24}, after all their opening local chunks."""
    visits = []
    for c in range(NPAIR):
        if c in (8, 16, 24):
            for vc in range(NVC):
                if 8 * vc + 8 <= c:
                    for i in range(c, c + 8):
                        visits.append(("vert", vc, i))
        for i in range(c, min(c + 8, NPAIR - 1) + 1):
            visits.append(("local", c, i))
    return visits


def vert_visit_order():
    return [(vc_, i_) for (kind, vc_, i_) in make_schedule() if kind == "vert"]


def vert_all_valid(vc, i):
    """True iff the (vc, i) vert visit is unmasked on EVERY core (r=0..7):
    highest slot kb = 16vc + 8 + r <= qb_min - 16 = 2i - 16 for r=7."""
    return 16 * vc + 15 <= 2 * i - 16


def vert_mask_slots():
    """Mask-table slot index per masked vert visit, keyed by (vc, i)."""
    slots = {}
    for vc, i in vert_visit_order():
        if not vert_all_valid(vc, i):
            slots[(vc, i)] = len(slots)
    return slots


_PROGRAM = None


def _build_program(loop_n=None, ablate=(), pv_delay=3, group=GROUP,
                   pt_bufs=None, exp_split=1, stage_bufs=2, oacc_bufs=4,
                   mask_eng="dve", store_eng="sp", ob_bufs=3, rd_bufs=4,
                   g0split=2):
    if pt_bufs is None:
        pt_bufs = pv_delay + 2
    """Build the SPMD program. loop_n: wrap the whole body (incl. input DMA)
    in an in-NEFF For loop with that trip count — used only for timing.
    ablate: subset of {"masks","pv","epi","exp"} — drop stages (timing only).
    pv_delay: groups of software-pipeline delay between S^T and PV."""
    import contextlib
    import concourse.bass as bass
    import concourse.mybir as mybir
    import concourse.tile as tile
    from concourse import bacc

    fp32 = mybir.dt.float32
    bf16 = mybir.dt.bfloat16

    nc = bacc.Bacc("TRN2", target_bir_lowering=False, debug=False, num_devices=H)

    qt_d = nc.dram_tensor("qt", [D, S], bf16, kind="ExternalInput").ap()
    kt_d = nc.dram_tensor("kt", [D, S], bf16, kind="ExternalInput").ap()
    ktv_d = nc.dram_tensor("ktv", [D, NVSLOT * BLK], bf16, kind="ExternalInput").ap()
    vaug_d = nc.dram_tensor("vaug", [128, NPAIR, D + 1], bf16, kind="ExternalInput").ap()
    vvaug_d = nc.dram_tensor("vvaug", [128, NVC, D + 1], bf16, kind="ExternalInput").ap()
    nvm = len(vert_mask_slots())
    vm_d = nc.dram_tensor("vm", [128, nvm, 128], bf16, kind="ExternalInput").ap()
    # lead = [kt[:,0:128] | qt[:,0:512]] packed by the host: the group-0
    # critical path rides ONE DMA chain instead of two serialized ones
    lead_d = nc.dram_tensor("lead", [D, 640], bf16, kind="ExternalInput").ap()
    tri_d = nc.dram_tensor("tri", [128, 128], bf16, kind="ExternalInput").ap()
    mstart_d = nc.dram_tensor("mstart", [128, 128], bf16, kind="ExternalInput").ap()
    # partition-major output: o[p, i, d] = out[i*128 + p, d]; host transposes
    o_d = nc.dram_tensor("o", [128, NPAIR, D], fp32, kind="ExternalOutput").ap()

    visits = make_schedule()
    # first/last visit index per pair
    first = {}
    last = {}
    for g, (kind, idx, i) in enumerate(visits):
        first.setdefault(i, g)
        last[i] = g
    # PSUM start_tensor_calc zeroes the full 2KB bank (zero-region), so only
    # the first matmul touching an oacc tile may carry start=True.
    tile_first = {}
    for g, (kind, idx, i) in enumerate(visits):
        tile_first.setdefault(i // 3, g)
    with tile.TileContext(nc) as tc:
        with (
            tc.tile_pool(name="big", bufs=1) as big,
            tc.tile_pool(name="stage", bufs=stage_bufs, space="PSUM") as stagep,
            tc.tile_pool(name="oacc", bufs=oacc_bufs, space="PSUM") as oaccp,
            tc.tile_pool(name="pt", bufs=pt_bufs) as ptp,
            tc.tile_pool(name="ob", bufs=ob_bufs) as obp,
            tc.tile_pool(name="rd", bufs=rd_bufs) as rdp,
        ):
            if loop_n is not None:
                loop_cm = tc.For_i(
                    0,
                    loop_n,
                    hint_engines=(
                        mybir.EngineType.PE,
                        mybir.EngineType.DVE,
                        mybir.EngineType.Activation,
                        mybir.EngineType.Pool,
                        mybir.EngineType.SP,
                    ),
                )
            else:
                loop_cm = contextlib.nullcontext()
            with loop_cm:
                _emit_body(nc, tc, locals(), frozenset(ablate),
                           pv_delay=pv_delay, group=group, exp_split=exp_split,
                           mask_eng=mask_eng, store_eng=store_eng,
                           g0split=g0split)
    nc.compile()
    return nc


def _emit_body(nc, tc, env, ablate=frozenset(), pv_delay=1, group=GROUP,
               exp_split=1, mask_eng="pool", store_eng="sp", g0split=3):
    GROUP = group
    import concourse.mybir as mybir

    fp32 = mybir.dt.float32
    bf16 = mybir.dt.bfloat16
    big, stagep, oaccp, ptp, obp, rdp = (
        env["big"], env["stagep"], env["oaccp"], env["ptp"], env["obp"], env["rdp"]
    )
    qt_d, kt_d, ktv_d, vaug_d, vvaug_d, vm_d, tri_d, mstart_d, lead_d, o_d = (
        env["qt_d"], env["kt_d"], env["ktv_d"], env["vaug_d"], env["vvaug_d"],
        env["vm_d"], env["tri_d"], env["mstart_d"], env["lead_d"], env["o_d"],
    )
    vm_slots = vert_mask_slots()
    visits, first, last, tile_first = (
        env["visits"], env["first"], env["last"], env["tile_first"]
    )
    n_groups = (len(visits) + GROUP - 1) // GROUP

    qt = big.tile([D, S], bf16)
    kt = big.tile([D, S], bf16)
    ktv = big.tile([D, NVSLOT * BLK], bf16)
    vaug = big.tile([128, NPAIR, D + 1], bf16)
    vvaug = big.tile([128, NVC, D + 1], bf16)
    vm = big.tile([128, len(vert_mask_slots()), 128], bf16)
    tri = big.tile([128, 128], bf16)
    mstart = big.tile([128, 128], bf16)
    lead = big.tile([D, 640], bf16)

    mask_e = nc.gpsimd if mask_eng == "pool" else nc.vector
    store_e = nc.sync if store_eng == "sp" else nc.gpsimd

    # Input DMAs spread over the three DMA-capable queues.  Transfers
    # serialize per queue, so each queue is ordered by first use; the
    # group-0-gating pieces (qt lead slices on SP, kt lead slices on ACT,
    # tri via Pool) resolve ~2.5-3.5us in.
    nc.sync.dma_start(out=lead[:], in_=lead_d[:])
    nc.sync.dma_start(out=qt[:, 512:1024], in_=qt_d[:, 512:1024])
    nc.sync.dma_start(out=qt[:, 1024:2048], in_=qt_d[:, 1024:2048])
    nc.sync.dma_start(out=kt[:, 512:1536], in_=kt_d[:, 512:1536])
    nc.sync.dma_start(out=qt[:, 2048:4096], in_=qt_d[:, 2048:4096])
    nc.sync.dma_start(out=kt[:, 1536:2816], in_=kt_d[:, 1536:2816])
    nc.sync.dma_start(out=kt[:, 2816:4096], in_=kt_d[:, 2816:4096])
    nc.sync.dma_start(out=vm[:], in_=vm_d[:])
    nc.sync.dma_start(out=ktv[:], in_=ktv_d[:])
    nc.sync.dma_start(out=vvaug[:], in_=vvaug_d[:])
    # vaug tails ride the idle SP queue end (needed ~14us/~24us in) rather
    # than the ACT queue, where their issue slots would delay exp dispatch.
    nc.sync.dma_start(out=vaug[:, 8:16], in_=vaug_d[:, 8:16])
    nc.sync.dma_start(out=vaug[:, 16:NPAIR], in_=vaug_d[:, 16:NPAIR])
    # Pool SWDGE: small group-0/1 tensors (tri gates masks(0)).
    nc.gpsimd.dma_start(out=tri[:], in_=tri_d[:])
    nc.gpsimd.dma_start(out=mstart[:], in_=mstart_d[:])
    # ACT HWDGE: kt chunks 1-3 (chunk 0 arrives inside lead) and the first
    # vaug third (first use is PV, pv_delay groups in).
    nc.scalar.dma_start(out=kt[:, 128:512], in_=kt_d[:, 128:512])
    nc.scalar.dma_start(out=vaug[:, 0:8], in_=vaug_d[:, 0:8])

    oacc_tiles = {}  # pair-group -> psum tile
    pending_pv = []  # software pipeline: PV of group gi-d emitted
    # after S^T of group gi so PE streams while ACT/DVE process gi-d

    for gi in range(n_groups):
        gvis = visits[gi * GROUP : (gi + 1) * GROUP]
        n = len(gvis)
        stage = stagep.tile([128, GROUP * 128], fp32, tag="stage")
        ptt = ptp.tile([128, GROUP * 128], bf16, tag="pt")

        # --- S^T matmuls, batched over runs of consecutive pairs
        # sharing one k-chunk, split at 4-slot (one PSUM bank) bounds.
        # start=True only on the first run per bank (bank zero-region).
        s = 0
        seen_banks = set()
        while s < n:
            kind, idx, i0 = gvis[s]
            e = s + 1
            while (
                e < n
                and e % 4 != 0
                and not (gi == 0 and e == 2)
                and gvis[e][0] == kind
                and gvis[e][1] == idx
                and gvis[e][2] == gvis[e - 1][2] + 1
                # keep runs on one side of the lead/qt boundary (pair 3|4)
                and not (kind == "local" and gvis[e][2] == 4 and i0 < 4)
            ):
                e += 1
            ln = e - s
            if kind == "local" and idx == 0:
                lhsT = lead[:, 0:128]
            elif kind == "local":
                lhsT = kt[:, idx * 128 : (idx + 1) * 128]
            else:
                lhsT = ktv[:, idx * 128 : (idx + 1) * 128]
            if kind == "local" and i0 + ln <= 4:
                rhs = lead[:, 128 + i0 * 128 : 128 + (i0 + ln) * 128]
            else:
                rhs = qt[:, i0 * 128 : (i0 + ln) * 128]
            bank = s // 4
            nc.tensor.matmul(
                stage[:, s * 128 : e * 128],
                lhsT,
                rhs,
                start=bank not in seen_banks,
                stop=True,
                skip_group_check=True,
            )
            if "dup_st" in ablate:
                nc.tensor.matmul(
                    stage[:, s * 128 : e * 128],
                    lhsT,
                    rhs,
                    start=False,
                    stop=True,
                    skip_group_check=True,
                )
            seen_banks.add(bank)
            s = e

        # software pipeline with end-taper: drain pending PV batches faster
        # near the tail so they overlap the last exps instead of following.
        remaining = n_groups - 1 - gi
        while len(pending_pv) > min(pv_delay - 1, max(0, remaining - 1)):
            pending_pv.pop(0)()

        # --- exp for the group (group 0 split finely so the first call only
        # needs the first two S^T slots -> earlier ACT rampup)
        if "exp" not in ablate:
            if gi == 0 and g0split == 3:
                bounds = [(0, 2), (2, 4), (4, n)]
            elif gi == 0 and g0split == 2:
                bounds = [(0, 4), (4, n)]
            else:
                per = (n + exp_split - 1) // exp_split
                bounds = [(es, min(n, es + per)) for es in range(0, n, per)]
            for es, ee in bounds:
                for _rep in range(2 if "dup_exp" in ablate else 1):
                    nc.scalar.activation(
                        out=ptt[:, es * 128 : ee * 128],
                        in_=stage[:, es * 128 : ee * 128],
                        func=mybir.ActivationFunctionType.Exp,
                    )

        # --- masks (tri/mstart on mask_e; vert validity = one full-tile
        # multiply on DVE, and only for visits not valid on every core)
        for s, (kind, idx, i) in enumerate(gvis):
            if "masks" in ablate:
                continue
            sl = slice(s * 128, (s + 1) * 128)
            reps = 2 if "dup_masks" in ablate else 1
            if kind == "local" and idx == i:
                for _rep in range(reps):
                    mask_e.tensor_mul(ptt[:, sl], ptt[:, sl], tri[:])
            elif kind == "local" and idx == i - 8:
                for _rep in range(reps):
                    mask_e.tensor_mul(ptt[:, sl], ptt[:, sl], mstart[:])
            elif kind == "vert" and (idx, i) in vm_slots:
                for _rep in range(reps):
                    nc.vector.tensor_mul(
                        ptt[:, sl], ptt[:, sl], vm[:, vm_slots[(idx, i)]]
                    )

        # --- PV matmuls + epilogue (deferred pv_delay groups)
        # oacc tile = [128, 2, 512] f32 (2 banks); pair slot jj = i % PG sits
        # at bank jj//3, cols (jj%3)*129 .. +129 so no slot crosses a bank.
        def pslot(oacc, i, a, b):
            jj = i % PG
            c0 = (jj % 3) * 129
            return oacc[:, jj // 3, c0 + a : c0 + b]

        def make_pv(gi, gvis, ptt):
            def emit_pv():
                if "pv" in ablate:
                    return
                for s, (kind, idx, i) in enumerate(gvis):
                    g = gi * GROUP + s
                    pg = i // PG
                    if pg not in oacc_tiles:
                        oacc_tiles[pg] = oaccp.tile(
                            [128, (PG + 2) // 3, 512], fp32, tag="oacc",
                            name=f"oacc{pg}"
                        )
                    oacc = oacc_tiles[pg]
                    rhs = vaug[:, idx] if kind == "local" else vvaug[:, idx]
                    nc.tensor.matmul(
                        pslot(oacc, i, 0, D + 1),
                        ptt[:, s * 128 : (s + 1) * 128],
                        rhs,
                        start=(g == tile_first[i // 3]),
                        stop=(g == last[i]) and "dup_pv" not in ablate,
                        skip_group_check=True,
                    )
                    if "dup_pv" in ablate:
                        nc.tensor.matmul(
                            pslot(oacc, i, 0, D + 1),
                            ptt[:, s * 128 : (s + 1) * 128],
                            rhs,
                            start=False,
                            stop=(g == last[i]),
                            skip_group_check=True,
                        )
                    # epilogue once per oacc tile (after its last pair
                    # closes): per-pair reciprocal of the PSUM denominator
                    # column + tensor_scalar multiply straight out of PSUM
                    # into a [128, PG, D] staging tile, one batched store.
                    pg_pairs = list(range(PG * pg, min(PG * (pg + 1), NPAIR)))
                    last_pg = (NPAIR - 1) // PG
                    if "epi" in ablate:
                        pass
                    elif pg == last_pg and g == last[i]:
                        # tail: per-pair epilogue+store as each pair closes,
                        # so the final store chain is one pair deep.
                        ob = obp.tile([128, 1, D], fp32, tag="obt")
                        rd = rdp.tile([128, 1], fp32, tag="rd")
                        nc.vector.reciprocal(rd[:], pslot(oacc, i, D, D + 1))
                        nc.vector.tensor_scalar_mul(
                            ob[:, 0], pslot(oacc, i, 0, D), rd[:]
                        )
                        (nc.scalar if i == NPAIR - 1 else store_e).dma_start(
                            out=o_d[:, i : i + 1, :], in_=ob[:]
                        )
                    elif i == pg_pairs[-1] and g == last[i]:
                        npp = len(pg_pairs)
                        ob = obp.tile([128, PG, D], fp32, tag="ob")
                        rd = rdp.tile([128, PG], fp32, tag="rd")
                        # one strided reciprocal covers all denominators
                        nc.vector.reciprocal(
                            rd[:, 0:npp],
                            oacc[:, 0, D : D + (npp - 1) * 129 + 1 : 129],
                        )
                        for jj, p in enumerate(pg_pairs):
                            nc.vector.tensor_scalar_mul(
                                ob[:, jj], pslot(oacc, p, 0, D),
                                rd[:, jj : jj + 1]
                            )
                        store_e.dma_start(
                            out=o_d[:, PG * pg : PG * pg + npp, :],
                            in_=ob[:, 0:npp],
                        )
            return emit_pv

        pending_pv.append(make_pv(gi, gvis, ptt))
    for f in pending_pv:
        f()


def _get_program():
    global _PROGRAM
    if _PROGRAM is None:
        _PROGRAM = _build_program()
    return _PROGRAM


def _host_inputs(q, k, v, sm_scale):
    """Per-core input dicts (host-side shard + layout)."""
    q = np.asarray(q, dtype=np.float32)
    k = np.asarray(k, dtype=np.float32)
    v = np.asarray(v, dtype=np.float32)
    smv = float(np.asarray(sm_scale, dtype=np.float32))

    tri = np.zeros((128, 128), dtype=BF16)
    p = np.arange(128)
    tri[p[:, None] <= p[None, :]] = BF16(1.0)
    mstart = np.zeros((128, 128), dtype=BF16)
    mstart[64:, :64] = BF16(1.0)

    vm_slots = vert_mask_slots()
    ins = []
    for h in range(H):
        r = 7 - h
        qh, kh, vh = q[0, h], k[0, h], v[0, h]
        qt = np.ascontiguousarray(qh.T).astype(BF16)
        # sm_scale baked into K so the device exp needs no scale operand
        kt = np.ascontiguousarray(kh.T * smv).astype(BF16)
        vblocks = [8 * j + r for j in range(NVSLOT)]
        kv = np.concatenate([kh[b * BLK : (b + 1) * BLK] for b in vblocks], axis=0)
        ktv = np.ascontiguousarray(kv.T * smv).astype(BF16)
        vaug = np.concatenate(
            [vh, np.ones((S, 1), np.float32)], axis=1
        ).astype(BF16)  # [4096, 129]
        vaug = np.ascontiguousarray(
            vaug.reshape(NPAIR, 128, D + 1).transpose(1, 0, 2)
        )  # [128, 32, 129]
        vv = np.concatenate([vh[b * BLK : (b + 1) * BLK] for b in vblocks], axis=0)
        vvaug = np.concatenate([vv, np.ones((NVSLOT * BLK, 1), np.float32)], axis=1)
        vvaug = np.ascontiguousarray(
            vvaug.astype(BF16).reshape(NVC, 128, D + 1).transpose(1, 0, 2)
        )  # [128, 3, 129]

        # per-visit full vert-validity mask tiles (only for visits that are
        # not all-valid on every core): vm[k, slot, q] = kb(k) <= qb(q) - 16
        vmt = np.zeros((128, len(vm_slots), 128), dtype=BF16)
        for (vc, i), si in vm_slots.items():
            kb = 8 * (2 * vc + (p >= 64).astype(np.int64)) + r      # [128] per k
            qb = 2 * i + (p >= 64).astype(np.int64)                 # [128] per q
            vmt[:, si, :] = (kb[:, None] <= qb[None, :] - 16).astype(BF16)
        lead = np.ascontiguousarray(
            np.concatenate([kt[:, 0:128], qt[:, 0:512]], axis=1)
        )
        ins.append(
            dict(
                qt=qt, kt=kt, ktv=ktv, vaug=vaug, vvaug=vvaug,
                vm=vmt, tri=tri, mstart=mstart, lead=lead,
            )
        )
    return ins


def kernel(q, k, v, sm_scale):
    from concourse.bass_utils import run_bass_kernel_spmd

    nc = _get_program()
    ins = _host_inputs(q, k, v, sm_scale)
    res = run_bass_kernel_spmd(nc, ins, core_ids=list(range(H)))
    # device output is partition-major o[p, i, d]; restore [S, D]
    out = np.stack(
        [
            res.results[h]["o"].transpose(1, 0, 2).reshape(S, D)
            for h in range(H)
        ],
        axis=0,
    )[None]
    return out.astype(np.float32)
